# revision 4
# baseline (speedup 1.0000x reference)
"""Additive (Bahdanau) attention on 8 trn2 NeuronCores via Bass/Tile.

Reference computation (per batch b):
  q = queries @ Wq                     [NQ, H]
  k = keys @ Wk                        [NK, H]
  scores[i,j] = w_v . tanh(q[i] + k[j])          [NQ, NK]
  attn = softmax(mask(scores, valid_lens[b]))    (mask -> -1e6)
  out = attn @ values                  [NQ, D]

Shapes: B=4, NQ=256, NK=1024, D=256, H=256, f32 inputs.

Sharding: 8 cores = (batch b = core//2) x (query half = core%2), so each
core handles nq=128 queries x nk=1024 keys, fully independent (no
collectives). Weights replicated.

Per-core dataflow (all on-chip; the [nq, nk, H] intermediate is tiled
through SBUF and never touches HBM):
  prologue: PE-transpose queries/keys, project to qT [h, q] / kT [h, k]
            (h on partitions).
  main loop over q in groups of G:
    DVE  tensor_scalar_add: feat[h, k] = kT[h, :] + qT[h, q]  (broadcast
         along free dim; f32 in -> bf16 out)
    ACT  one jumbo tanh instruction per group (the throughput bound:
         33.5M tanh / 128 lanes / 1.2 GHz ~ 220 us)
    PE   per (q, ktile, htile): matmul lhsT=tanh_feat[128h,128k],
         rhs=w_v column -> scoresT column in PSUM [k, q]
  epilogue: mask during PSUM->SBUF copy (per-partition scalar add),
            PE-transpose to [q, k], masked softmax (max/exp/sum/recip),
            PE-transpose attn, attn @ V, scale by 1/sum, DMA out.

reps>1 builds a timing variant with the whole compute wrapped in a
device-side For_i loop (identical work each iteration).
"""

import numpy as np
import ml_dtypes

import concourse.bass as bass
import concourse.mybir as mybir
import concourse.tile as tile
import concourse.bacc as bacc
from concourse.bass_utils import run_bass_kernel_spmd

F32 = mybir.dt.float32
BF16 = mybir.dt.bfloat16
BF = ml_dtypes.bfloat16

B, NQ, NK, D, H = 4, 256, 1024, 256, 256
NCORES = 8
NQS = NQ // 2          # queries per core
KT = NK // 128         # 8 k-tiles
NEG = -1e6

Tanh = mybir.ActivationFunctionType.Tanh
Exp = mybir.ActivationFunctionType.Exp

_CACHE = {}


def _build_nc(reps=1, g=4):
    nc = bacc.Bacc("TRN2", target_bir_lowering=False, debug=False,
                   num_devices=NCORES)

    q_d = nc.dram_tensor("q_bf", [NQS, D], BF16, kind="ExternalInput")
    keys_d = nc.dram_tensor("keys_bf", [NK, D], BF16, kind="ExternalInput")
    vals_d = nc.dram_tensor("vals_bf", [NK, D], BF16, kind="ExternalInput")
    wq_d = nc.dram_tensor("wq_bf", [D, H], BF16, kind="ExternalInput")
    wk_d = nc.dram_tensor("wk_bf", [D, H], BF16, kind="ExternalInput")
    wv_d = nc.dram_tensor("wv_cols", [128, 2], BF16, kind="ExternalInput")
    maskT_d = nc.dram_tensor("maskT", [128, KT], F32, kind="ExternalInput")
    idb_d = nc.dram_tensor("idb", [128, 128], BF16, kind="ExternalInput")
    idf_d = nc.dram_tensor("idf", [128, 128], F32, kind="ExternalInput")
    out_d = nc.dram_tensor("out", [NQS, D], F32, kind="ExternalOutput")

    with tile.TileContext(nc) as tc:
        with tc.tile_pool(name="const", bufs=1) as cp:
            t_q = cp.tile([128, D], BF16, tag="q", name="q")
            t_keys = [cp.tile([128, D], BF16, tag=f"k{i}", name=f"k{i}") for i in range(KT)]
            t_vals = [cp.tile([128, D], BF16, tag=f"v{i}", name=f"v{i}") for i in range(KT)]
            t_wq = [cp.tile([128, H], BF16, tag=f"wq{d}", name=f"wq{d}") for d in range(2)]
            t_wk = [cp.tile([128, H], BF16, tag=f"wk{d}", name=f"wk{d}") for d in range(2)]
            t_wv = cp.tile([128, 2], BF16, tag="wv", name="wv")
            t_maskT = cp.tile([128, KT], F32, tag="maskT", name="maskT")
            t_idb = cp.tile([128, 128], BF16, tag="idb", name="idb")
            t_idf = cp.tile([128, 128], F32, tag="idf", name="idf")

            nc.sync.dma_start(t_q[:], q_d[:])
            for i in range(KT):
                nc.sync.dma_start(t_keys[i][:], keys_d[i * 128:(i + 1) * 128, :])
                nc.sync.dma_start(t_vals[i][:], vals_d[i * 128:(i + 1) * 128, :])
            for d in range(2):
                nc.sync.dma_start(t_wq[d][:], wq_d[d * 128:(d + 1) * 128, :])
                nc.sync.dma_start(t_wk[d][:], wk_d[d * 128:(d + 1) * 128, :])
            nc.sync.dma_start(t_wv[:], wv_d[:])
            nc.sync.dma_start(t_maskT[:], maskT_d[:])
            nc.sync.dma_start(t_idb[:], idb_d[:])
            nc.sync.dma_start(t_idf[:], idf_d[:])

            # persistent SBUF intermediates
            qrT = [cp.tile([128, NQS], BF16, tag=f"qrT{d}", name=f"qrT{d}") for d in range(2)]
            keysT = [cp.tile([128, NK], BF16, tag=f"keysT{d}", name=f"keysT{d}") for d in range(2)]
            qT = [cp.tile([128, NQS], F32, tag=f"qT{h}", name=f"qT{h}") for h in range(2)]
            kT = [cp.tile([128, NK], F32, tag=f"kT{h}", name=f"kT{h}") for h in range(2)]
            scoresT = cp.tile([128, NK], F32, tag="scoresT", name="scoresT")
            exp_sb = cp.tile([128, NK], BF16, tag="exp", name="exp")
            attnT = [cp.tile([128, 128], BF16, tag=f"aT{i}", name=f"aT{i}") for i in range(KT)]
            mx = cp.tile([128, 1], F32, tag="mx", name="mx")
            negmx = cp.tile([128, 1], F32, tag="negmx", name="negmx")
            ssum = cp.tile([128, 1], F32, tag="ssum", name="ssum")
            rcp = cp.tile([128, 1], F32, tag="rcp", name="rcp")
            out_sb = cp.tile([128, D], F32, tag="out", name="out")

            # ---- prologue: transposes ----
            with tc.tile_pool(name="ppt", bufs=4, space="PSUM") as ppt:
                for d in range(2):
                    pt = ppt.tile([128, 128], BF16, tag="pt", name="pt")
                    nc.tensor.transpose(pt[:], t_q[:, d * 128:(d + 1) * 128], t_idb[:])
                    nc.vector.tensor_copy(qrT[d][:], pt[:])
                for i in range(KT):
                    for d in range(2):
                        pt = ppt.tile([128, 128], BF16, tag="pt", name="pt")
                        nc.tensor.transpose(pt[:], t_keys[i][:, d * 128:(d + 1) * 128], t_idb[:])
                        nc.vector.tensor_copy(keysT[d][:, i * 128:(i + 1) * 128], pt[:])

            # ---- prologue: projections (transposed layout, h on partitions) ----
            with tc.tile_pool(name="ppj", bufs=2, space="PSUM") as ppj:
                for h in range(2):
                    pq = ppj.tile([128, NQS], F32, tag="pq", name="pq")
                    for d in range(2):
                        nc.tensor.matmul(pq[:], t_wq[d][:, h * 128:(h + 1) * 128],
                                         qrT[d][:], start=(d == 0), stop=(d == 1))
                    nc.vector.tensor_copy(qT[h][:], pq[:])
                for h in range(2):
                    for c in range(2):
                        pk = ppj.tile([128, 512], F32, tag="pk", name="pk")
                        for d in range(2):
                            nc.tensor.matmul(pk[:], t_wk[d][:, h * 128:(h + 1) * 128],
                                             keysT[d][:, c * 512:(c + 1) * 512],
                                             start=(d == 0), stop=(d == 1))
                        nc.vector.tensor_copy(kT[h][:, c * 512:(c + 1) * 512], pk[:])

            with (
                tc.tile_pool(name="psT", bufs=1, space="PSUM") as psTp,
                tc.tile_pool(name="feat", bufs=3) as fp,
                tc.tile_pool(name="tanh", bufs=3) as tp,
                tc.tile_pool(name="psS", bufs=1, space="PSUM") as psSp,
                tc.tile_pool(name="psE", bufs=2, space="PSUM") as psEp,
                tc.tile_pool(name="psO", bufs=1, space="PSUM") as psOp,
            ):
                psT = [psTp.tile([128, 512], F32, tag=f"psT{i}", name=f"psT{i}") for i in range(2)]
                psS = psSp.tile([128, NK], F32, tag="psS", name="psS")

                def body():
                    # ---- main loop: feat/tanh/matvec ----
                    for gi in range(NQS // g):
                        f = fp.tile([128, g * 2 * NK], BF16, tag="f", name="f")
                        for i in range(g):
                            q = gi * g + i
                            for h in range(2):
                                nc.vector.tensor_scalar_add(
                                    f[:, (i * 2 + h) * NK:(i * 2 + h + 1) * NK],
                                    kT[h][:], qT[h][:, q:q + 1])
                        t = tp.tile([128, g * 2 * NK], BF16, tag="t", name="t")
                        nc.scalar.activation(t[:], f[:], Tanh)
                        for i in range(g):
                            q = gi * g + i
                            for kt in range(KT):
                                dst = psT[kt // 4]
                                col = (kt % 4) * 128 + q
                                for h in range(2):
                                    nc.tensor.matmul(
                                        dst[:, col:col + 1],
                                        t[:, (i * 2 + h) * NK + kt * 128:(i * 2 + h) * NK + (kt + 1) * 128],
                                        t_wv[:, h:h + 1],
                                        start=(h == 0), stop=(h == 1))

                    # ---- mask during PSUM->SBUF copy ----
                    for kt in range(KT):
                        src = psT[kt // 4]
                        off = (kt % 4) * 128
                        nc.vector.tensor_scalar_add(
                            scoresT[:, kt * 128:(kt + 1) * 128],
                            src[:, off:off + 128], t_maskT[:, kt:kt + 1])

                    # ---- softmax ----
                    for kt in range(KT):
                        nc.tensor.transpose(psS[:, kt * 128:(kt + 1) * 128],
                                            scoresT[:, kt * 128:(kt + 1) * 128], t_idf[:])
                    nc.vector.reduce_max(mx[:], psS[:], axis=mybir.AxisListType.X)
                    nc.vector.tensor_scalar_mul(negmx[:], mx[:], -1.0)
                    nc.scalar.activation(exp_sb[:], psS[:], Exp, bias=negmx[:, 0:1])
                    nc.vector.reduce_sum(ssum[:], exp_sb[:], axis=mybir.AxisListType.X)
                    nc.vector.reciprocal(rcp[:], ssum[:])

                    # ---- attn @ V ----
                    for kt in range(KT):
                        pe = psEp.tile([128, 128], BF16, tag="pe", name="pe")
                        nc.tensor.transpose(pe[:], exp_sb[:, kt * 128:(kt + 1) * 128], t_idb[:])
                        nc.vector.tensor_copy(attnT[kt][:], pe[:])
                    po = psOp.tile([128, D], F32, tag="po", name="po")
                    for kt in range(KT):
                        nc.tensor.matmul(po[:], attnT[kt][:], t_vals[kt][:],
                                         start=(kt == 0), stop=(kt == KT - 1))
                    nc.vector.tensor_scalar_mul(out_sb[:], po[:], rcp[:, 0:1])

                if reps == 1:
                    body()
                else:
                    with tc.For_i(0, reps, 1):
                        body()

            nc.sync.dma_start(out_d[:], out_sb[:])

    nc.compile()
    return nc


def get_nc(reps=1, g=4):
    key = (reps, g)
    if key not in _CACHE:
        _CACHE[key] = _build_nc(reps, g)
    return _CACHE[key]


def shard_inputs(queries, keys, values, valid_lens, Wq, Wk, w_v):
    ids_b = np.eye(128, dtype=BF)
    ids_f = np.eye(128, dtype=np.float32)
    wv_cols = np.ascontiguousarray(w_v.astype(BF).reshape(2, 128).T)
    wq_bf = Wq.astype(BF)
    wk_bf = Wk.astype(BF)
    in_maps = []
    for c in range(NCORES):
        b, qh = c // 2, c % 2
        m = np.where(np.arange(NK) < valid_lens[b], 0.0, NEG).astype(np.float32)
        in_maps.append({
            "q_bf": queries[b, qh * NQS:(qh + 1) * NQS].astype(BF),
            "keys_bf": keys[b].astype(BF),
            "vals_bf": values[b].astype(BF),
            "wq_bf": wq_bf,
            "wk_bf": wk_bf,
            "wv_cols": wv_cols,
            "maskT": np.ascontiguousarray(m.reshape(KT, 128).T),
            "idb": ids_b,
            "idf": ids_f,
        })
    return in_maps


def assemble(results):
    out = np.empty((B, NQ, D), dtype=np.float32)
    for c in range(NCORES):
        b, qh = c // 2, c % 2
        out[b, qh * NQS:(qh + 1) * NQS] = results[c]["out"]
    return out


def kernel(queries, keys, values, valid_lens, Wq, Wk, w_v):
    nc = get_nc()
    in_maps = shard_inputs(queries, keys, values, valid_lens, Wq, Wk, w_v)
    res = run_bass_kernel_spmd(nc, in_maps, list(range(NCORES)))
    return assemble(res.results)


# revision 5
# speedup vs baseline: 1.3935x; 1.3935x over previous
"""Additive (Bahdanau) attention on 8 trn2 NeuronCores via Bass/Tile.

Reference computation (per batch b):
  q = queries @ Wq                     [NQ, H]
  k = keys @ Wk                        [NK, H]
  scores[i,j] = w_v . tanh(q[i] + k[j])          [NQ, NK]
  attn = softmax(mask(scores, valid_lens[b]))    (mask -> -1e6)
  out = attn @ values                  [NQ, D]

Shapes: B=4, NQ=256, NK=1024, D=256, H=256, f32 inputs.

Sharding: 8 cores = (batch b = core//2) x (query half = core%2), so each
core handles nq=128 queries x nk=1024 keys, fully independent (no
collectives). Weights replicated.

Per-core dataflow (all on-chip; the [nq, nk, H] intermediate is tiled
through SBUF and never touches HBM):
  prologue: PE-transpose queries/keys, project to qT [h, q] / kT [h, k]
            (h on partitions).
  main loop over q in groups of G:
    DVE  tensor_scalar_add: feat[h, k] = kT[h, :] + qT[h, q]  (broadcast
         along free dim; f32 in -> bf16 out)
    ACT  one jumbo tanh instruction per group (the throughput bound:
         33.5M tanh / 128 lanes / 1.2 GHz ~ 220 us)
    PE   per (q, ktile, htile): matmul lhsT=tanh_feat[128h,128k],
         rhs=w_v column -> scoresT column in PSUM [k, q]
  epilogue: mask during PSUM->SBUF copy (per-partition scalar add),
            PE-transpose to [q, k], masked softmax (max/exp/sum/recip),
            PE-transpose attn, attn @ V, scale by 1/sum, DMA out.

reps>1 builds a timing variant with the whole compute wrapped in a
device-side For_i loop (identical work each iteration).
"""

import numpy as np
import ml_dtypes

import concourse.bass as bass
import concourse.mybir as mybir
import concourse.tile as tile
import concourse.bacc as bacc
from concourse.bass_utils import run_bass_kernel_spmd

F32 = mybir.dt.float32
BF16 = mybir.dt.bfloat16
BF = ml_dtypes.bfloat16

B, NQ, NK, D, H = 4, 256, 1024, 256, 256
NCORES = 8
NQS = NQ // 2          # queries per core
KT = NK // 128         # 8 k-tiles
NEG = -1e6

Tanh = mybir.ActivationFunctionType.Tanh
Exp = mybir.ActivationFunctionType.Exp

_CACHE = {}


def _build_nc(reps=1, g=4, nke=NK):
    kte = nke // 128
    nc = bacc.Bacc("TRN2", target_bir_lowering=False, debug=False,
                   num_devices=NCORES)

    q_d = nc.dram_tensor("q_bf", [NQS, D], BF16, kind="ExternalInput")
    keys_d = nc.dram_tensor("keys_bf", [nke, D], BF16, kind="ExternalInput")
    vals_d = nc.dram_tensor("vals_bf", [nke, D], BF16, kind="ExternalInput")
    wq_d = nc.dram_tensor("wq_bf", [D, H], BF16, kind="ExternalInput")
    wk_d = nc.dram_tensor("wk_bf", [D, H], BF16, kind="ExternalInput")
    wv_d = nc.dram_tensor("wv_cols", [128, 2], BF16, kind="ExternalInput")
    maskT_d = nc.dram_tensor("maskT", [128, kte], F32, kind="ExternalInput")
    idb_d = nc.dram_tensor("idb", [128, 128], BF16, kind="ExternalInput")
    idf_d = nc.dram_tensor("idf", [128, 128], F32, kind="ExternalInput")
    out_d = nc.dram_tensor("out", [NQS, D], F32, kind="ExternalOutput")

    with tile.TileContext(nc) as tc:
        with tc.tile_pool(name="const", bufs=1) as cp:
            t_q = cp.tile([128, D], BF16, tag="q", name="q")
            t_keys = [cp.tile([128, D], BF16, tag=f"k{i}", name=f"k{i}") for i in range(kte)]
            t_vals = [cp.tile([128, D], BF16, tag=f"v{i}", name=f"v{i}") for i in range(kte)]
            t_wq = [cp.tile([128, H], BF16, tag=f"wq{d}", name=f"wq{d}") for d in range(2)]
            t_wk = [cp.tile([128, H], BF16, tag=f"wk{d}", name=f"wk{d}") for d in range(2)]
            t_wv = cp.tile([128, 2], BF16, tag="wv", name="wv")
            t_maskT = cp.tile([128, kte], F32, tag="maskT", name="maskT")
            t_idb = cp.tile([128, 128], BF16, tag="idb", name="idb")
            t_idf = cp.tile([128, 128], F32, tag="idf", name="idf")

            nc.sync.dma_start(t_q[:], q_d[:])
            for i in range(kte):
                nc.sync.dma_start(t_keys[i][:], keys_d[i * 128:(i + 1) * 128, :])
                nc.sync.dma_start(t_vals[i][:], vals_d[i * 128:(i + 1) * 128, :])
            for d in range(2):
                nc.sync.dma_start(t_wq[d][:], wq_d[d * 128:(d + 1) * 128, :])
                nc.sync.dma_start(t_wk[d][:], wk_d[d * 128:(d + 1) * 128, :])
            nc.sync.dma_start(t_wv[:], wv_d[:])
            nc.sync.dma_start(t_maskT[:], maskT_d[:])
            nc.sync.dma_start(t_idb[:], idb_d[:])
            nc.sync.dma_start(t_idf[:], idf_d[:])

            # persistent SBUF intermediates
            qrT = [cp.tile([128, NQS], BF16, tag=f"qrT{d}", name=f"qrT{d}") for d in range(2)]
            keysT = [cp.tile([128, nke], BF16, tag=f"keysT{d}", name=f"keysT{d}") for d in range(2)]
            qT = [cp.tile([128, NQS], F32, tag=f"qT{h}", name=f"qT{h}") for h in range(2)]
            kT = [cp.tile([128, nke], F32, tag=f"kT{h}", name=f"kT{h}") for h in range(2)]
            scoresT = cp.tile([128, nke], F32, tag="scoresT", name="scoresT")
            exp_sb = cp.tile([128, nke], BF16, tag="exp", name="exp")
            attnT = [cp.tile([128, 128], BF16, tag=f"aT{i}", name=f"aT{i}") for i in range(kte)]
            mx = cp.tile([128, 1], F32, tag="mx", name="mx")
            negmx = cp.tile([128, 1], F32, tag="negmx", name="negmx")
            ssum = cp.tile([128, 1], F32, tag="ssum", name="ssum")
            rcp = cp.tile([128, 1], F32, tag="rcp", name="rcp")
            out_sb = cp.tile([128, D], F32, tag="out", name="out")

            # ---- prologue: transposes ----
            with tc.tile_pool(name="ppt", bufs=4, space="PSUM") as ppt:
                for d in range(2):
                    pt = ppt.tile([128, 128], BF16, tag="pt", name="pt")
                    nc.tensor.transpose(pt[:], t_q[:, d * 128:(d + 1) * 128], t_idb[:])
                    nc.vector.tensor_copy(qrT[d][:], pt[:])
                for i in range(kte):
                    for d in range(2):
                        pt = ppt.tile([128, 128], BF16, tag="pt", name="pt")
                        nc.tensor.transpose(pt[:], t_keys[i][:, d * 128:(d + 1) * 128], t_idb[:])
                        nc.vector.tensor_copy(keysT[d][:, i * 128:(i + 1) * 128], pt[:])

            # ---- prologue: projections (transposed layout, h on partitions) ----
            with tc.tile_pool(name="ppj", bufs=2, space="PSUM") as ppj:
                for h in range(2):
                    pq = ppj.tile([128, NQS], F32, tag="pq", name="pq")
                    for d in range(2):
                        nc.tensor.matmul(pq[:], t_wq[d][:, h * 128:(h + 1) * 128],
                                         qrT[d][:], start=(d == 0), stop=(d == 1))
                    nc.vector.tensor_copy(qT[h][:], pq[:])
                kchunks = [(c0, min(c0 + 512, nke)) for c0 in range(0, nke, 512)]
                for h in range(2):
                    for (c0, c1) in kchunks:
                        pk = ppj.tile([128, 512], F32, tag="pk", name="pk")
                        for d in range(2):
                            nc.tensor.matmul(pk[:, 0:c1 - c0], t_wk[d][:, h * 128:(h + 1) * 128],
                                             keysT[d][:, c0:c1],
                                             start=(d == 0), stop=(d == 1))
                        nc.vector.tensor_copy(kT[h][:, c0:c1], pk[:, 0:c1 - c0])

            with (
                tc.tile_pool(name="psT", bufs=1, space="PSUM") as psTp,
                tc.tile_pool(name="feat", bufs=3) as fp,
                tc.tile_pool(name="tanh", bufs=3) as tp,
                tc.tile_pool(name="psS", bufs=1, space="PSUM") as psSp,
                tc.tile_pool(name="psE", bufs=2, space="PSUM") as psEp,
                tc.tile_pool(name="psO", bufs=1, space="PSUM") as psOp,
            ):
                psT = [psTp.tile([128, 512], F32, tag=f"psT{i}", name=f"psT{i}") for i in range((kte + 3) // 4)]
                psS = psSp.tile([128, nke], F32, tag="psS", name="psS")

                def body():
                    # ---- main loop: feat/tanh/matvec ----
                    for gi in range(NQS // g):
                        f = fp.tile([128, g * 2 * nke], BF16, tag="f", name="f")
                        for i in range(g):
                            q = gi * g + i
                            for h in range(2):
                                nc.vector.tensor_scalar_add(
                                    f[:, (i * 2 + h) * nke:(i * 2 + h + 1) * nke],
                                    kT[h][:], qT[h][:, q:q + 1])
                        t = tp.tile([128, g * 2 * nke], BF16, tag="t", name="t")
                        nc.scalar.activation(t[:], f[:], Tanh)
                        for i in range(g):
                            q = gi * g + i
                            for kt in range(kte):
                                dst = psT[kt // 4]
                                col = (kt % 4) * 128 + q
                                for h in range(2):
                                    nc.tensor.matmul(
                                        dst[:, col:col + 1],
                                        t[:, (i * 2 + h) * nke + kt * 128:(i * 2 + h) * nke + (kt + 1) * 128],
                                        t_wv[:, h:h + 1],
                                        start=(h == 0), stop=(h == 1))

                    # ---- mask during PSUM->SBUF copy ----
                    for kt in range(kte):
                        src = psT[kt // 4]
                        off = (kt % 4) * 128
                        nc.vector.tensor_scalar_add(
                            scoresT[:, kt * 128:(kt + 1) * 128],
                            src[:, off:off + 128], t_maskT[:, kt:kt + 1])

                    # ---- softmax ----
                    for kt in range(kte):
                        nc.tensor.transpose(psS[:, kt * 128:(kt + 1) * 128],
                                            scoresT[:, kt * 128:(kt + 1) * 128], t_idf[:])
                    nc.vector.reduce_max(mx[:], psS[:], axis=mybir.AxisListType.X)
                    nc.vector.tensor_scalar_mul(negmx[:], mx[:], -1.0)
                    nc.scalar.activation(exp_sb[:], psS[:], Exp, bias=negmx[:, 0:1])
                    nc.vector.reduce_sum(ssum[:], exp_sb[:], axis=mybir.AxisListType.X)
                    nc.vector.reciprocal(rcp[:], ssum[:])

                    # ---- attn @ V ----
                    for kt in range(kte):
                        pe = psEp.tile([128, 128], BF16, tag="pe", name="pe")
                        nc.tensor.transpose(pe[:], exp_sb[:, kt * 128:(kt + 1) * 128], t_idb[:])
                        nc.vector.tensor_copy(attnT[kt][:], pe[:])
                    po = psOp.tile([128, D], F32, tag="po", name="po")
                    for kt in range(kte):
                        nc.tensor.matmul(po[:], attnT[kt][:], t_vals[kt][:],
                                         start=(kt == 0), stop=(kt == kte - 1))
                    nc.vector.tensor_scalar_mul(out_sb[:], po[:], rcp[:, 0:1])

                if reps == 1:
                    body()
                else:
                    with tc.For_i(0, reps, 1):
                        body()

            nc.sync.dma_start(out_d[:], out_sb[:])

    nc.compile()
    return nc


def get_nc(reps=1, g=4, nke=NK):
    key = (reps, g, nke)
    if key not in _CACHE:
        _CACHE[key] = _build_nc(reps, g, nke)
    return _CACHE[key]


def nk_eff(valid_lens):
    return int(128 * ((int(np.max(valid_lens)) + 127) // 128))


def shard_inputs(queries, keys, values, valid_lens, Wq, Wk, w_v):
    nke = nk_eff(valid_lens)
    kte = nke // 128
    ids_b = np.eye(128, dtype=BF)
    ids_f = np.eye(128, dtype=np.float32)
    wv_cols = np.ascontiguousarray(w_v.astype(BF).reshape(2, 128).T)
    wq_bf = Wq.astype(BF)
    wk_bf = Wk.astype(BF)
    in_maps = []
    for c in range(NCORES):
        b, qh = c // 2, c % 2
        m = np.where(np.arange(nke) < valid_lens[b], 0.0, NEG).astype(np.float32)
        in_maps.append({
            "q_bf": queries[b, qh * NQS:(qh + 1) * NQS].astype(BF),
            "keys_bf": keys[b, :nke].astype(BF),
            "vals_bf": values[b, :nke].astype(BF),
            "wq_bf": wq_bf,
            "wk_bf": wk_bf,
            "wv_cols": wv_cols,
            "maskT": np.ascontiguousarray(m.reshape(kte, 128).T),
            "idb": ids_b,
            "idf": ids_f,
        })
    return in_maps


def assemble(results):
    out = np.empty((B, NQ, D), dtype=np.float32)
    for c in range(NCORES):
        b, qh = c // 2, c % 2
        out[b, qh * NQS:(qh + 1) * NQS] = results[c]["out"]
    return out


def kernel(queries, keys, values, valid_lens, Wq, Wk, w_v):
    nc = get_nc(nke=nk_eff(valid_lens))
    in_maps = shard_inputs(queries, keys, values, valid_lens, Wq, Wk, w_v)
    res = run_bass_kernel_spmd(nc, in_maps, list(range(NCORES)))
    return assemble(res.results)
